# revision 7
# baseline (speedup 1.0000x reference)
"""Trainium2 Bass kernel for nn_Aggregator (GNN message-passing aggregator).

Computes, for COO graph (rows, cols, vals) over N nodes with D=256 features:
    side = segment_sum(vals[:,None] * ego[cols], rows, N)
    out  = lrelu((ego + side) @ W1 + b1) + lrelu((ego * side) @ W2 + b2)

Strategy (8 NeuronCores, row-sharded):
  * Each core owns N/8 output rows. Host partitions edges by destination row
    block (128 rows) and source chunk (32768 rows, so col indices fit int16).
  * On device, per block: gpsimd.dma_gather pulls ego[cols] rows from HBM into
    SBUF with edges laid out 128-per-partition-tile; the weighted segment-sum
    becomes a PE matmul with an on-the-fly one-hot selector S built by a single
    fused DVE tensor_scalar op per 128-edge tile ( (iota==row_local)*val ).
  * side accumulates in PSUM; the bi-interaction combine (two 256x256 matmuls,
    bias via a K=1 ones-row matmul, LeakyReLU on ACT) is fused per block.
"""

import math

import numpy as np

P = 128
NCORES = 8
D = 256
CHUNK_BITS = 15  # source chunk size 32768 so col_local fits int16


def _plan(rows, cols, n, n_cores, g_blocks):
    """Static schedule shared by all cores: per-(block,chunk) capacities padded
    to 128 (max over cores), grouped gather segments, and ktile->block map."""
    rpc = n // n_cores
    nblk = (rpc + P - 1) // P
    ch_sz = 1 << CHUNK_BITS
    nch = (n + ch_sz - 1) // ch_sz

    core = rows // rpc
    ric = rows - core * rpc
    blk = ric >> 7
    ch = cols >> CHUNK_BITS

    lin = (core.astype(np.int64) * nblk + blk) * nch + ch
    counts = np.bincount(lin, minlength=n_cores * nblk * nch).reshape(
        n_cores, nblk, nch
    )
    caps = counts.max(axis=0)
    caps = ((caps + P - 1) // P) * P  # [nblk, nch], multiples of 128

    ngrp = (nblk + g_blocks - 1) // g_blocks
    seg_base = np.zeros((nblk, nch), dtype=np.int64)
    groups = []
    pos = 0
    for g in range(ngrp):
        blks = list(range(g * g_blocks, min((g + 1) * g_blocks, nblk)))
        gbase = pos
        chsegs = []  # (chunk, offset_in_group, length)
        ktmap = []  # per ktile in group: (local_block_idx, global_block)
        for c in range(nch):
            coff = pos - gbase
            clen = 0
            for bi, b in enumerate(blks):
                seg_base[b, c] = pos
                for _ in range(caps[b, c] // P):
                    ktmap.append((bi, b))
                pos += caps[b, c]
                clen += caps[b, c]
            if clen:
                chsegs.append((c, coff, clen))
        groups.append(dict(base=gbase, length=pos - gbase, chsegs=chsegs,
                           ktmap=ktmap, blocks=blks))
    return dict(rpc=rpc, nblk=nblk, nch=nch, ngrp=ngrp, groups=groups,
                seg_base=seg_base, caps=caps, tot=pos, g_blocks=g_blocks)


def _pack(rows, cols, vals, n, n_cores, plan):
    """Per-core packed streams: wrapped int16 gather indices, row_local (f32,
    -1 for padding) and val (f32, 0 for padding) per stream slot."""
    rpc = plan["rpc"]
    nblk = plan["nblk"]
    nch = plan["nch"]
    tot = plan["tot"]
    seg_base = plan["seg_base"]

    core = (rows // rpc).astype(np.int64)
    ric = rows - core * rpc
    blk = (ric >> 7).astype(np.int64)
    rloc = (ric & 127).astype(np.float32)
    ch = (cols >> CHUNK_BITS).astype(np.int64)
    cloc = (cols & ((1 << CHUNK_BITS) - 1)).astype(np.int16)

    lin = (core * nblk + blk) * nch + ch
    order = np.argsort(lin, kind="stable")
    sl = lin[order]
    # rank within segment
    starts = np.r_[0, np.flatnonzero(np.diff(sl)) + 1]
    seg_lens = np.diff(np.r_[starts, len(sl)])
    rank = np.arange(len(sl)) - np.repeat(starts, seg_lens)
    b_s = (sl // nch) % nblk
    c_s = sl % nch
    pos = seg_base[b_s, c_s] + rank
    core_s = sl // (nblk * nch)

    packed = []
    for ci in range(n_cores):
        m = core_s == ci
        idxs = np.zeros(tot, np.int16)
        rv = np.full(tot, -1.0, np.float32)
        vv = np.zeros(tot, np.float32)
        p_ = pos[m]
        o_ = order[m]
        idxs[p_] = cloc[o_]
        rv[p_] = rloc[o_]
        vv[p_] = vals[o_]
        idx_w = np.tile(np.ascontiguousarray(idxs.reshape(-1, 16).T), (8, 1))
        rv_w = np.ascontiguousarray(rv.reshape(-1, P).T)
        vv_w = np.ascontiguousarray(vv.reshape(-1, P).T)
        packed.append(dict(idx=idx_w, rowv=rv_w, valv=vv_w))
    return packed


def _build_nc(n, n_cores, plan, reps=1, max_gather=1024):
    import concourse.bacc as bacc
    import concourse.tile as tile
    import concourse.mybir as mybir
    from concourse.masks import make_identity

    dt = mybir.dt
    aot = mybir.AluOpType
    rpc = plan["rpc"]
    nblk = plan["nblk"]
    tot = plan["tot"]
    ch_sz = 1 << CHUNK_BITS
    rpc_pad = nblk * P

    nc = bacc.Bacc("TRN2", target_bir_lowering=False, debug=False,
                   num_devices=n_cores)
    ego_d = nc.dram_tensor("ego", [n, D], dt.float32, kind="ExternalInput")
    egom_d = nc.dram_tensor("egom", [rpc_pad, D], dt.float32, kind="ExternalInput")
    w1_d = nc.dram_tensor("w1", [D, D], dt.float32, kind="ExternalInput")
    w2_d = nc.dram_tensor("w2", [D, D], dt.float32, kind="ExternalInput")
    b1_d = nc.dram_tensor("b1", [1, D], dt.float32, kind="ExternalInput")
    b2_d = nc.dram_tensor("b2", [1, D], dt.float32, kind="ExternalInput")
    idx_d = nc.dram_tensor("idx", [P, tot // 16], dt.int16, kind="ExternalInput")
    rowv_d = nc.dram_tensor("rowv", [P, tot // P], dt.float32, kind="ExternalInput")
    valv_d = nc.dram_tensor("valv", [P, tot // P], dt.float32, kind="ExternalInput")
    reps_d = nc.dram_tensor("reps", [1, 1], dt.int32, kind="ExternalInput")
    out_d = nc.dram_tensor("out", [rpc_pad, D], dt.float32, kind="ExternalOutput")

    max_gkt = max(g["length"] for g in plan["groups"]) // P

    with tile.TileContext(nc) as tc:
        with tc.tile_pool(name="const", bufs=1) as cp, \
             tc.tile_pool(name="stream", bufs=2) as sp, \
             tc.tile_pool(name="work", bufs=3) as wp, \
             tc.tile_pool(name="psum", bufs=2, space="PSUM") as pp:
            # constants
            w1 = cp.tile([P, 2, D], dt.float32)
            w2 = cp.tile([P, 2, D], dt.float32)
            for kh in range(2):
                nc.sync.dma_start(out=w1[:, kh, :], in_=w1_d.ap()[kh * P:(kh + 1) * P, :])
                nc.sync.dma_start(out=w2[:, kh, :], in_=w2_d.ap()[kh * P:(kh + 1) * P, :])
            b1 = cp.tile([1, D], dt.float32)
            nc.sync.dma_start(out=b1[:], in_=b1_d.ap())
            b2 = cp.tile([1, D], dt.float32)
            nc.sync.dma_start(out=b2[:], in_=b2_d.ap())
            ones1 = cp.tile([1, P], dt.float32)
            nc.vector.memset(ones1[:], 1.0)
            iota_i = cp.tile([P, P], dt.int32)
            nc.gpsimd.iota(iota_i[:], pattern=[[1, P]], base=0, channel_multiplier=0)
            iota_f = cp.tile([P, P], dt.float32)
            nc.vector.tensor_copy(out=iota_f[:], in_=iota_i[:])
            ident = cp.tile([P, P], dt.float32)
            make_identity(nc, ident[:])
            reps_sb = cp.tile([1, 1], dt.int32)
            nc.sync.dma_start(out=reps_sb[:], in_=reps_d.ap())

            def body():
                for g in plan["groups"]:
                    gbase, glen = g["base"], g["length"]
                    gkt = glen // P
                    gB = len(g["blocks"])
                    idx_sb = sp.tile([P, max_gkt * 8], dt.int16, tag="idx")
                    nc.sync.dma_start(
                        out=idx_sb[:, :glen // 16],
                        in_=idx_d.ap()[:, gbase // 16:(gbase + glen) // 16])
                    rowv = sp.tile([P, max_gkt], dt.float32, tag="rowv")
                    nc.sync.dma_start(
                        out=rowv[:, :gkt],
                        in_=rowv_d.ap()[:, gbase // P:(gbase + glen) // P])
                    valv = sp.tile([P, max_gkt], dt.float32, tag="valv")
                    nc.sync.dma_start(
                        out=valv[:, :gkt],
                        in_=valv_d.ap()[:, gbase // P:(gbase + glen) // P])

                    msgs = sp.tile([P, max_gkt, D], dt.float32, tag="msgs")
                    for (c, coff, clen) in g["chsegs"]:
                        lo = c * ch_sz
                        hi = min((c + 1) * ch_sz, n)
                        for soff in range(0, clen, max_gather):
                            slen = min(max_gather, clen - soff)
                            o = coff + soff
                            nc.gpsimd.dma_gather(
                                out_ap=msgs[:, o // P:(o + slen) // P, :],
                                in_ap=ego_d.ap()[lo:hi, :],
                                idxs_ap=idx_sb[:, o // 16:(o + slen) // 16],
                                num_idxs=slen, num_idxs_reg=slen, elem_size=D)

                    s_tiles = sp.tile([P, max_gkt, P], dt.float32, tag="s")
                    for j in range(gkt):
                        nc.vector.tensor_scalar(
                            out=s_tiles[:, j, :], in0=iota_f[:],
                            scalar1=rowv[:, j:j + 1], scalar2=valv[:, j:j + 1],
                            op0=aot.is_equal, op1=aot.mult)

                    side_ps = pp.tile([P, len(g["blocks"]), D], dt.float32,
                                      space="PSUM", tag="side")
                    for bi in range(gB):
                        kts = [j for j, (bj, _) in enumerate(g["ktmap"]) if bj == bi]
                        if not kts:
                            nc.vector.memset(side_ps[:, bi, :], 0.0)
                            continue
                        for i, j in enumerate(kts):
                            nc.tensor.matmul(
                                out=side_ps[:, bi, :], lhsT=s_tiles[:, j, :],
                                rhs=msgs[:, j, :], start=(i == 0),
                                stop=(i == len(kts) - 1))

                    for bi, b in enumerate(g["blocks"]):
                        ego_sb = wp.tile([P, D], dt.float32, tag="egob")
                        nc.sync.dma_start(out=ego_sb[:],
                                          in_=egom_d.ap()[b * P:(b + 1) * P, :])
                        a_sb = wp.tile([P, D], dt.float32, tag="a")
                        m_sb = wp.tile([P, D], dt.float32, tag="m")
                        nc.vector.tensor_tensor(out=a_sb[:], in0=side_ps[:, bi, :],
                                                in1=ego_sb[:], op=aot.add)
                        nc.vector.tensor_tensor(out=m_sb[:], in0=side_ps[:, bi, :],
                                                in1=ego_sb[:], op=aot.mult)
                        tr_ps = pp.tile([P, 4, P], dt.float32, space="PSUM", tag="tr")
                        amT = wp.tile([P, 4, P], dt.float32, tag="amT")
                        for kh in range(2):
                            nc.tensor.transpose(out=tr_ps[:, kh, :],
                                                in_=a_sb[:, kh * P:(kh + 1) * P],
                                                identity=ident[:])
                            nc.tensor.transpose(out=tr_ps[:, 2 + kh, :],
                                                in_=m_sb[:, kh * P:(kh + 1) * P],
                                                identity=ident[:])
                        for q in range(4):
                            nc.scalar.copy(out=amT[:, q, :], in_=tr_ps[:, q, :])
                        h_ps = pp.tile([P, 2, D], dt.float32, space="PSUM", tag="h")
                        for kh in range(2):
                            nc.tensor.matmul(out=h_ps[:, 0, :], lhsT=amT[:, kh, :],
                                             rhs=w1[:, kh, :], start=(kh == 0),
                                             stop=False)
                        nc.tensor.matmul(out=h_ps[:, 0, :], lhsT=ones1[:], rhs=b1[:],
                                         start=False, stop=True)
                        for kh in range(2):
                            nc.tensor.matmul(out=h_ps[:, 1, :], lhsT=amT[:, 2 + kh, :],
                                             rhs=w2[:, kh, :], start=(kh == 0),
                                             stop=False)
                        nc.tensor.matmul(out=h_ps[:, 1, :], lhsT=ones1[:], rhs=b2[:],
                                         start=False, stop=True)
                        r1 = wp.tile([P, D], dt.float32, tag="r1")
                        r2 = wp.tile([P, D], dt.float32, tag="r2")
                        nc.scalar.activation(out=r1[:], in_=h_ps[:, 0, :],
                                             func=mybir.ActivationFunctionType.Lrelu,
                                             alpha=0.01)
                        nc.scalar.activation(out=r2[:], in_=h_ps[:, 1, :],
                                             func=mybir.ActivationFunctionType.Lrelu,
                                             alpha=0.01)
                        outb = wp.tile([P, D], dt.float32, tag="outb")
                        nc.vector.tensor_tensor(out=outb[:], in0=r1[:], in1=r2[:],
                                                op=aot.add)
                        nc.sync.dma_start(out=out_d.ap()[b * P:(b + 1) * P, :],
                                          in_=outb[:])

            if reps > 1:
                with tc.For_i(0, reps, 1):
                    body()
            else:
                body()
    nc.compile()
    return nc


def _run(nc, in_maps, n_cores):
    from concourse.bass_utils import run_bass_kernel_spmd
    res = run_bass_kernel_spmd(nc, in_maps, core_ids=list(range(n_cores)))
    return res.results


def _prepare(ego, vals, W1, b1, W2, b2, rows, cols, n, n_cores, g_blocks,
             reps=1):
    plan = _plan(rows, cols, n, n_cores, g_blocks)
    packed = _pack(rows, cols, vals, n, n_cores, plan)
    rpc = plan["rpc"]
    rpc_pad = plan["nblk"] * P
    ego = np.ascontiguousarray(ego, dtype=np.float32)
    W1 = np.ascontiguousarray(W1, dtype=np.float32)
    W2 = np.ascontiguousarray(W2, dtype=np.float32)
    b1 = np.ascontiguousarray(b1, dtype=np.float32).reshape(1, D)
    b2 = np.ascontiguousarray(b2, dtype=np.float32).reshape(1, D)
    reps_arr = np.array([[reps]], dtype=np.int32)
    in_maps = []
    for c in range(n_cores):
        egom = np.zeros((rpc_pad, D), np.float32)
        egom[:rpc] = ego[c * rpc:(c + 1) * rpc]
        in_maps.append(dict(
            ego=ego, egom=egom, w1=W1, w2=W2, b1=b1, b2=b2,
            idx=packed[c]["idx"], rowv=packed[c]["rowv"],
            valv=packed[c]["valv"], reps=reps_arr))
    return plan, in_maps


def kernel(ego_embeddings, vals, W1, b1, W2, b2, rows, cols):
    ego = np.asarray(ego_embeddings, dtype=np.float32)
    vals = np.asarray(vals, dtype=np.float32)
    W1 = np.asarray(W1)
    W2 = np.asarray(W2)
    b1 = np.asarray(b1)
    b2 = np.asarray(b2)
    rows = np.asarray(rows).astype(np.int64)
    cols = np.asarray(cols).astype(np.int64)
    n = ego.shape[0]
    g_blocks = 2

    plan, in_maps = _prepare(ego, vals, W1, b1, W2, b2, rows, cols, n, NCORES,
                             g_blocks)
    nc = _build_nc(n, NCORES, plan)
    results = _run(nc, in_maps, NCORES)
    rpc = plan["rpc"]
    out = np.concatenate([results[c]["out"][:rpc] for c in range(NCORES)],
                         axis=0)
    return out.astype(np.float32)


if __name__ == "__main__":
    # small self-test
    rng = np.random.default_rng(0)
    n, e = NCORES * 1024, 16384
    ego = rng.standard_normal((n, D)).astype(np.float32)
    rows = rng.integers(0, n, e).astype(np.int32)
    cols = rng.integers(0, n, e).astype(np.int32)
    vals = rng.random(e, dtype=np.float32)
    W1 = (rng.standard_normal((D, D)) / 16).astype(np.float32)
    W2 = (rng.standard_normal((D, D)) / 16).astype(np.float32)
    b1 = np.zeros(D, np.float32)
    b2 = np.zeros(D, np.float32)
    got = kernel(ego, vals, W1, b1, W2, b2, rows, cols)
    side = np.zeros((n, D), np.float32)
    np.add.at(side, rows, vals[:, None] * ego[cols])
    lr = lambda x: np.where(x > 0, x, 0.01 * x)
    exp = lr((ego + side) @ W1 + b1) + lr((ego * side) @ W2 + b2)
    err = np.abs(got - exp).max() / np.abs(exp).max()
    print("rel err:", err)


# revision 19
# speedup vs baseline: 1.8273x; 1.8273x over previous
"""Trainium2 Bass kernel for nn_Aggregator (GNN message-passing aggregator).

Computes, for COO graph (rows, cols, vals) over N nodes with D=256 features:
    side = segment_sum(vals[:,None] * ego[cols], rows, N)
    out  = lrelu((ego + side) @ W1 + b1) + lrelu((ego * side) @ W2 + b2)

Strategy (8 NeuronCores, row-sharded):
  * Each core owns N/8 output rows. Host partitions edges by destination row
    block (128 rows) and source chunk (32768 rows, so col indices fit int16).
  * On device, per block: gpsimd.dma_gather pulls ego[cols] rows from HBM into
    SBUF with edges laid out 128-per-partition-tile; the weighted segment-sum
    becomes a PE matmul with an on-the-fly one-hot selector S built by a single
    fused DVE tensor_scalar op per 128-edge tile ( (iota==row_local)*val ).
  * side accumulates in PSUM; the bi-interaction combine (two 256x256 matmuls,
    bias via a K=1 ones-row matmul, LeakyReLU on ACT) is fused per block.
"""

import math

import numpy as np

P = 128
NCORES = 8
D = 256
CHUNK_BITS = 15  # source chunk size 32768 so col_local fits int16


def _plan(rows, cols, n, n_cores, g_blocks):
    """Static schedule shared by all cores: per-(block,chunk) capacities padded
    to 128 (max over cores), grouped gather segments, and ktile->block map."""
    rpc = n // n_cores
    nblk = (rpc + P - 1) // P
    ch_sz = 1 << CHUNK_BITS
    nch = (n + ch_sz - 1) // ch_sz

    core = rows // rpc
    ric = rows - core * rpc
    blk = ric >> 7
    ch = cols >> CHUNK_BITS

    lin = (core.astype(np.int64) * nblk + blk) * nch + ch
    counts = np.bincount(lin, minlength=n_cores * nblk * nch).reshape(
        n_cores, nblk, nch
    )
    counts_per_core = counts
    caps = counts.max(axis=0)
    caps = ((caps + P - 1) // P) * P  # [nblk, nch], multiples of 128

    ngrp = (nblk + g_blocks - 1) // g_blocks
    seg_base = np.zeros((nblk, nch), dtype=np.int64)
    groups = []
    pos = 0
    for g in range(ngrp):
        blks = list(range(g * g_blocks, min((g + 1) * g_blocks, nblk)))
        gbase = pos
        chsegs = []  # (chunk, offset_in_group, length)
        ktmap = []  # per ktile in group: (local_block_idx, global_block)
        for c in range(nch):
            coff = pos - gbase
            clen = 0
            for bi, b in enumerate(blks):
                seg_base[b, c] = pos
                for _ in range(caps[b, c] // P):
                    ktmap.append((bi, b))
                pos += caps[b, c]
                clen += caps[b, c]
            if clen:
                chsegs.append((c, coff, clen))
        groups.append(dict(base=gbase, length=pos - gbase, chsegs=chsegs,
                           ktmap=ktmap, blocks=blks))
    return dict(rpc=rpc, nblk=nblk, nch=nch, ngrp=ngrp, groups=groups,
                seg_base=seg_base, caps=caps, tot=pos, g_blocks=g_blocks,
                counts=counts_per_core)


def _pack(rows, cols, vals, n, n_cores, plan):
    """Per-core packed streams: wrapped int16 gather indices, row_local (f32,
    -1 for padding) and val (f32, 0 for padding) per stream slot."""
    rpc = plan["rpc"]
    nblk = plan["nblk"]
    nch = plan["nch"]
    tot = plan["tot"]
    seg_base = plan["seg_base"]
    caps = plan["caps"]
    counts_per_core = plan["counts"]

    core = (rows // rpc).astype(np.int64)
    ric = rows - core * rpc
    blk = (ric >> 7).astype(np.int64)
    rloc = (ric & 127).astype(np.float32)
    ch = (cols >> CHUNK_BITS).astype(np.int64)
    cloc = (cols & ((1 << CHUNK_BITS) - 1)).astype(np.int16)

    # secondary sort by source col: ascending addresses within each gather
    # segment give the SDMA engines / HBM row buffers locality.
    lin = (core * nblk + blk) * nch + ch
    order = np.lexsort((cols, lin))
    sl = lin[order]
    # rank within segment
    starts = np.r_[0, np.flatnonzero(np.diff(sl)) + 1]
    seg_lens = np.diff(np.r_[starts, len(sl)])
    rank = np.arange(len(sl)) - np.repeat(starts, seg_lens)
    b_s = (sl // nch) % nblk
    c_s = sl % nch
    pos = seg_base[b_s, c_s] + rank
    core_s = sl // (nblk * nch)

    packed = []
    for ci in range(n_cores):
        m = core_s == ci
        idxs = np.zeros(tot, np.int16)
        rv = np.full(tot, -1.0, np.float32)
        vv = np.zeros(tot, np.float32)
        p_ = pos[m]
        o_ = order[m]
        idxs[p_] = cloc[o_]
        rv[p_] = rloc[o_]
        vv[p_] = vals[o_]
        # fill per-segment padding with the last real col of the segment:
        # duplicate reads of a just-read HBM row are much cheaper than
        # random reads of row 0.
        cnts = counts_per_core[ci].reshape(-1)
        sb_flat = seg_base.reshape(-1)
        caps_flat = caps.reshape(-1)
        for si in range(len(sb_flat)):
            cap = caps_flat[si]
            cnt = cnts[si]
            if cap > cnt and cnt > 0:
                s = sb_flat[si]
                idxs[s + cnt:s + cap] = idxs[s + cnt - 1]
        idx_w = np.tile(np.ascontiguousarray(idxs.reshape(-1, 16).T), (8, 1))
        rv_w = np.ascontiguousarray(rv.reshape(-1, P).T)
        vv_w = np.ascontiguousarray(vv.reshape(-1, P).T)
        packed.append(dict(idx=idx_w, rowv=rv_w, valv=vv_w))
    return packed


def _build_nc(n, n_cores, plan, reps=1, max_gather=1024, mode='full',
              nq=1, single_packet=True, scratch=16384,
              sbufs=2, wbufs=3, pbufs=2):
    import concourse.bacc as bacc
    import concourse.tile as tile
    import concourse.mybir as mybir
    from concourse.masks import make_identity

    dt = mybir.dt
    aot = mybir.AluOpType
    rpc = plan["rpc"]
    nblk = plan["nblk"]
    tot = plan["tot"]
    ch_sz = 1 << CHUNK_BITS
    rpc_pad = nblk * P

    nc = bacc.Bacc("TRN2", target_bir_lowering=False, debug=False,
                   num_devices=n_cores, num_swdge_queues=nq,
                   dynamic_dma_scratch_size=scratch)
    ego_d = nc.dram_tensor("ego", [n, D], dt.float32, kind="ExternalInput")
    egom_d = nc.dram_tensor("egom", [rpc_pad, D], dt.float32, kind="ExternalInput")
    w1_d = nc.dram_tensor("w1", [D, D], dt.float32, kind="ExternalInput")
    w2_d = nc.dram_tensor("w2", [D, D], dt.float32, kind="ExternalInput")
    b1_d = nc.dram_tensor("b1", [1, D], dt.float32, kind="ExternalInput")
    b2_d = nc.dram_tensor("b2", [1, D], dt.float32, kind="ExternalInput")
    idx_d = nc.dram_tensor("idx", [P, tot // 16], dt.int16, kind="ExternalInput")
    rowv_d = nc.dram_tensor("rowv", [P, tot // P], dt.float32, kind="ExternalInput")
    valv_d = nc.dram_tensor("valv", [P, tot // P], dt.float32, kind="ExternalInput")
    reps_d = nc.dram_tensor("reps", [1, 1], dt.int32, kind="ExternalInput")
    out_d = nc.dram_tensor("out", [rpc_pad, D], dt.float32, kind="ExternalOutput")

    max_gkt = max(g["length"] for g in plan["groups"]) // P

    with tile.TileContext(nc) as tc:
        with tc.tile_pool(name="const", bufs=1) as cp, \
             tc.tile_pool(name="stream", bufs=sbufs) as sp, \
             tc.tile_pool(name="work", bufs=wbufs) as wp, \
             tc.tile_pool(name="psum", bufs=pbufs, space="PSUM") as pp:
            # constants
            w1 = cp.tile([P, 2, D], dt.float32)
            w2 = cp.tile([P, 2, D], dt.float32)
            for kh in range(2):
                nc.sync.dma_start(out=w1[:, kh, :], in_=w1_d.ap()[kh * P:(kh + 1) * P, :])
                nc.sync.dma_start(out=w2[:, kh, :], in_=w2_d.ap()[kh * P:(kh + 1) * P, :])
            b1 = cp.tile([1, D], dt.float32)
            nc.sync.dma_start(out=b1[:], in_=b1_d.ap())
            b2 = cp.tile([1, D], dt.float32)
            nc.sync.dma_start(out=b2[:], in_=b2_d.ap())
            ones1 = cp.tile([1, P], dt.float32)
            nc.vector.memset(ones1[:], 1.0)
            iota_i = cp.tile([P, P], dt.int32)
            nc.gpsimd.iota(iota_i[:], pattern=[[1, P]], base=0, channel_multiplier=0)
            iota_f = cp.tile([P, P], dt.float32)
            nc.vector.tensor_copy(out=iota_f[:], in_=iota_i[:])
            ident = cp.tile([P, P], dt.float32)
            make_identity(nc, ident[:])
            reps_sb = cp.tile([1, 1], dt.int32)
            nc.sync.dma_start(out=reps_sb[:], in_=reps_d.ap())
            if mode == 'compute':
                for _ in range(2):
                    mz = sp.tile([P, max_gkt, D], dt.float32, tag="msgs")
                    nc.vector.memset(mz[:], 0.0)
            if mode == 'gather':
                tick_ps = pp.tile([P, P], dt.float32, space="PSUM", tag="tick")
            else:
                tick_ps = None

            def body():
                for g in plan["groups"]:
                    gbase, glen = g["base"], g["length"]
                    gkt = glen // P
                    gB = len(g["blocks"])
                    idx_sb = sp.tile([P, max_gkt * 8], dt.int16, tag="idx")
                    nc.sync.dma_start(
                        out=idx_sb[:, :glen // 16],
                        in_=idx_d.ap()[:, gbase // 16:(gbase + glen) // 16])
                    rowv = sp.tile([P, max_gkt], dt.float32, tag="rowv")
                    nc.sync.dma_start(
                        out=rowv[:, :gkt],
                        in_=rowv_d.ap()[:, gbase // P:(gbase + glen) // P])
                    valv = sp.tile([P, max_gkt], dt.float32, tag="valv")
                    nc.sync.dma_start(
                        out=valv[:, :gkt],
                        in_=valv_d.ap()[:, gbase // P:(gbase + glen) // P])

                    msgs = sp.tile([P, max_gkt, D], dt.float32, tag="msgs")
                    gi = 0
                    if mode != 'compute':
                        for (c, coff, clen) in g["chsegs"]:
                            lo = c * ch_sz
                            hi = min((c + 1) * ch_sz, n)
                            for soff in range(0, clen, max_gather):
                                slen = min(max_gather, clen - soff)
                                o = coff + soff
                                nc.gpsimd.dma_gather(
                                    out_ap=msgs[:, o // P:(o + slen) // P, :],
                                    in_ap=ego_d.ap()[lo:hi, :],
                                    idxs_ap=idx_sb[:, o // 16:(o + slen) // 16],
                                    num_idxs=slen, num_idxs_reg=slen,
                                    elem_size=D, single_packet=single_packet,
                                    queue_num=gi % nq)
                                gi += 1
                    if mode == 'gather':
                        tk = wp.tile([1, 4], dt.float32, tag="tk")
                        nc.vector.memset(tk[:], 0.0)
                        tk2 = wp.tile([1, 4], dt.float32, tag="tk2")
                        nc.scalar.copy(out=tk2[:], in_=tk[:])
                        nc.tensor.matmul(out=tick_ps[:], lhsT=ones1[:],
                                         rhs=ones1[:], start=True, stop=True)
                        continue

                    s_tiles = sp.tile([P, max_gkt, P], dt.float32, tag="s")
                    for j in range(gkt):
                        nc.vector.tensor_scalar(
                            out=s_tiles[:, j, :], in0=iota_f[:],
                            scalar1=rowv[:, j:j + 1], scalar2=valv[:, j:j + 1],
                            op0=aot.is_equal, op1=aot.mult)

                    side_ps = pp.tile([P, len(g["blocks"]), D], dt.float32,
                                      space="PSUM", tag="side")
                    for bi in range(gB):
                        kts = [j for j, (bj, _) in enumerate(g["ktmap"]) if bj == bi]
                        if not kts:
                            nc.vector.memset(side_ps[:, bi, :], 0.0)
                            continue
                        for i, j in enumerate(kts):
                            nc.tensor.matmul(
                                out=side_ps[:, bi, :], lhsT=s_tiles[:, j, :],
                                rhs=msgs[:, j, :], start=(i == 0),
                                stop=(i == len(kts) - 1))

                    for bi, b in enumerate(g["blocks"]):
                        ego_sb = wp.tile([P, D], dt.float32, tag="egob")
                        nc.scalar.dma_start(out=ego_sb[:],
                                            in_=egom_d.ap()[b * P:(b + 1) * P, :])
                        a_sb = wp.tile([P, D], dt.float32, tag="a")
                        m_sb = wp.tile([P, D], dt.float32, tag="m")
                        nc.vector.tensor_tensor(out=a_sb[:], in0=side_ps[:, bi, :],
                                                in1=ego_sb[:], op=aot.add)
                        nc.vector.tensor_tensor(out=m_sb[:], in0=side_ps[:, bi, :],
                                                in1=ego_sb[:], op=aot.mult)
                        tr_ps = pp.tile([P, 4, P], dt.float32, space="PSUM", tag="tr")
                        amT = wp.tile([P, 4, P], dt.float32, tag="amT")
                        for kh in range(2):
                            nc.tensor.transpose(out=tr_ps[:, kh, :],
                                                in_=a_sb[:, kh * P:(kh + 1) * P],
                                                identity=ident[:])
                            nc.tensor.transpose(out=tr_ps[:, 2 + kh, :],
                                                in_=m_sb[:, kh * P:(kh + 1) * P],
                                                identity=ident[:])
                        for q in range(4):
                            nc.scalar.copy(out=amT[:, q, :], in_=tr_ps[:, q, :])
                        h_ps = pp.tile([P, 2, D], dt.float32, space="PSUM", tag="h")
                        for kh in range(2):
                            nc.tensor.matmul(out=h_ps[:, 0, :], lhsT=amT[:, kh, :],
                                             rhs=w1[:, kh, :], start=(kh == 0),
                                             stop=False)
                        nc.tensor.matmul(out=h_ps[:, 0, :], lhsT=ones1[:], rhs=b1[:],
                                         start=False, stop=True)
                        for kh in range(2):
                            nc.tensor.matmul(out=h_ps[:, 1, :], lhsT=amT[:, 2 + kh, :],
                                             rhs=w2[:, kh, :], start=(kh == 0),
                                             stop=False)
                        nc.tensor.matmul(out=h_ps[:, 1, :], lhsT=ones1[:], rhs=b2[:],
                                         start=False, stop=True)
                        r1 = wp.tile([P, D], dt.float32, tag="r1")
                        r2 = wp.tile([P, D], dt.float32, tag="r2")
                        nc.scalar.activation(out=r1[:], in_=h_ps[:, 0, :],
                                             func=mybir.ActivationFunctionType.Lrelu,
                                             alpha=0.01)
                        nc.scalar.activation(out=r2[:], in_=h_ps[:, 1, :],
                                             func=mybir.ActivationFunctionType.Lrelu,
                                             alpha=0.01)
                        outb = wp.tile([P, D], dt.float32, tag="outb")
                        nc.vector.tensor_tensor(out=outb[:], in0=r1[:], in1=r2[:],
                                                op=aot.add)
                        nc.scalar.dma_start(out=out_d.ap()[b * P:(b + 1) * P, :],
                                            in_=outb[:])

            if reps > 1:
                with tc.For_i(0, reps, 1):
                    body()
            else:
                body()
    nc.compile()
    return nc


def _run(nc, in_maps, n_cores):
    from concourse.bass_utils import run_bass_kernel_spmd
    res = run_bass_kernel_spmd(nc, in_maps, core_ids=list(range(n_cores)))
    return res.results


def _prepare(ego, vals, W1, b1, W2, b2, rows, cols, n, n_cores, g_blocks,
             reps=1):
    plan = _plan(rows, cols, n, n_cores, g_blocks)
    packed = _pack(rows, cols, vals, n, n_cores, plan)
    rpc = plan["rpc"]
    rpc_pad = plan["nblk"] * P
    ego = np.ascontiguousarray(ego, dtype=np.float32)
    W1 = np.ascontiguousarray(W1, dtype=np.float32)
    W2 = np.ascontiguousarray(W2, dtype=np.float32)
    b1 = np.ascontiguousarray(b1, dtype=np.float32).reshape(1, D)
    b2 = np.ascontiguousarray(b2, dtype=np.float32).reshape(1, D)
    reps_arr = np.array([[reps]], dtype=np.int32)
    in_maps = []
    for c in range(n_cores):
        egom = np.zeros((rpc_pad, D), np.float32)
        egom[:rpc] = ego[c * rpc:(c + 1) * rpc]
        in_maps.append(dict(
            ego=ego, egom=egom, w1=W1, w2=W2, b1=b1, b2=b2,
            idx=packed[c]["idx"], rowv=packed[c]["rowv"],
            valv=packed[c]["valv"], reps=reps_arr))
    return plan, in_maps


def _kernel_once(ego, vals, W1, b1, W2, b2, rows, cols):
    n = ego.shape[0]
    plan, in_maps = _prepare(ego, vals, W1, b1, W2, b2, rows, cols, n, NCORES,
                             g_blocks=1)
    nc = _build_nc(n, NCORES, plan, nq=4, sbufs=3)
    results = _run(nc, in_maps, NCORES)
    rpc = plan["rpc"]
    out = np.concatenate([results[c]["out"][:rpc] for c in range(NCORES)],
                         axis=0)
    return out.astype(np.float32)


def kernel(ego_embeddings, vals, W1, b1, W2, b2, rows, cols):
    import os

    ego = np.asarray(ego_embeddings, dtype=np.float32)
    vals = np.asarray(vals, dtype=np.float32)
    W1 = np.asarray(W1, dtype=np.float32)
    W2 = np.asarray(W2, dtype=np.float32)
    b1 = np.asarray(b1, dtype=np.float32)
    b2 = np.asarray(b2, dtype=np.float32)
    rows = np.asarray(rows).astype(np.int64)
    cols = np.asarray(cols).astype(np.int64)

    try:
        return _kernel_once(ego, vals, W1, b1, W2, b2, rows, cols)
    except Exception as e:
        if os.environ.get("AGGK_NO_RETRY"):
            raise
        # The axon/neuron device occasionally lands in an unrecoverable
        # state; the in-process jax client cannot recover from it, so retry
        # the whole execution in fresh subprocesses.
        import subprocess
        import sys
        import tempfile
        import time as _time

        print(f"kernel: in-process run failed ({type(e).__name__}); "
              f"retrying in subprocess", file=sys.stderr)
        tmpdir = tempfile.mkdtemp()
        inp = os.path.join(tmpdir, "in.npz")
        outp = os.path.join(tmpdir, "out.npy")
        np.savez(inp, ego=ego, vals=vals, W1=W1, b1=b1, W2=W2, b2=b2,
                 rows=rows, cols=cols)
        prog = (
            "import numpy as np, importlib.util, sys\n"
            f"spec = importlib.util.spec_from_file_location('aggk', {__file__!r})\n"
            "m = importlib.util.module_from_spec(spec); spec.loader.exec_module(m)\n"
            f"d = np.load({inp!r})\n"
            "out = m._kernel_once(d['ego'], d['vals'], d['W1'], d['b1'],"
            " d['W2'], d['b2'], d['rows'], d['cols'])\n"
            f"np.save({outp!r}, out)\n"
        )
        env = dict(os.environ, AGGK_NO_RETRY="1")
        last = None
        for attempt in range(3):
            _time.sleep(20 * attempt)
            try:
                subprocess.run([sys.executable, "-c", prog], check=True,
                               env=env, timeout=3600)
                return np.load(outp)
            except Exception as e2:  # noqa: PERF203
                last = e2
                print(f"kernel: subprocess attempt {attempt} failed: {e2}",
                      file=sys.stderr)
        raise last


if __name__ == "__main__":
    # small self-test
    rng = np.random.default_rng(0)
    n, e = NCORES * 1024, 16384
    ego = rng.standard_normal((n, D)).astype(np.float32)
    rows = rng.integers(0, n, e).astype(np.int32)
    cols = rng.integers(0, n, e).astype(np.int32)
    vals = rng.random(e, dtype=np.float32)
    W1 = (rng.standard_normal((D, D)) / 16).astype(np.float32)
    W2 = (rng.standard_normal((D, D)) / 16).astype(np.float32)
    b1 = np.zeros(D, np.float32)
    b2 = np.zeros(D, np.float32)
    got = kernel(ego, vals, W1, b1, W2, b2, rows, cols)
    side = np.zeros((n, D), np.float32)
    np.add.at(side, rows, vals[:, None] * ego[cols])
    lr = lambda x: np.where(x > 0, x, 0.01 * x)
    exp = lr((ego + side) @ W1 + b1) + lr((ego * side) @ W2 + b2)
    err = np.abs(got - exp).max() / np.abs(exp).max()
    print("rel err:", err)
